# revision 23
# baseline (speedup 1.0000x reference)
"""Trainium2 Bass kernel for nn_DetectionLoss (YOLO-style detection loss).

Strategy (pure data parallel over 8 NeuronCores, 256 images each):
  - det loaded channel-major in 32-img chunks with a 128-partition
    overlapping-window AP (partition p reads channel p of each image, with
    p in [125,128) spilling into the next image's channels; the host pads
    the flat det buffer so the final image stays in bounds). 128-partition
    DMAs spread evenly over all 16 SDMA engines (~286 GB/s vs ~99 GB/s for
    the 125-partition layout).
  - One GPSIMD ap_gather per chunk pulls all 128 channel-partitions at the
    32 object cells per image -> G[128ch, obj].
  - PE transposes G into object-major GT[128obj, ch] tiles (ACT copies).
  - DVE does IoU / argmax / per-object loss terms in 2 pipelined passes;
    last-writer-wins dedup is deferred to a single [64-partition, 2048]
    pair-compare at the end.
  - Dense no-obj conf sum via a separate per-anchor reload of channel 4.
  - Output: per-core partial sums [128, 16]; host reduces across cores.
"""
import numpy as np

GRID = 13
NA = 5
NCLS = 20
CH = 25
NCH = NA * CH          # 125
CELLS = GRID * GRID    # 169
O = 32                 # objects per image
B = 2048               # global batch
NCORES = 8
BLOC = B // NCORES     # 256 images per core
C_IMG = 32             # images per chunk
NCHUNK = BLOC // C_IMG                   # 8
NE = C_IMG * CELLS                       # 5408 elems/partition per chunk
NIDX = C_IMG * O                         # 1024 gather idxs per chunk
NOBJ = BLOC * O                          # 8192 objects per core
J2 = NOBJ // 128                         # 64 object columns
IMFLT = NCH * CELLS                      # 21125 floats per image
DPAD = 1024                              # host-side pad floats on det
NPASS = 4
CPP = NCHUNK // NPASS                    # chunks per pass (2)
JPP = J2 // NPASS                        # j2 per pass (16)

ANCHORS = np.array([1.3221, 1.73145, 3.19275, 4.00944, 5.05587,
                    8.09892, 9.47112, 4.84053, 11.2364, 10.0071],
                   dtype=np.float32)

_CACHE = {}


def _make_consts():
    """Host-precomputed, data-independent constant input tensors."""
    consts = {}
    consts["c_ident"] = np.eye(128, dtype=np.float32)
    # 8 partition-selector matrices for the idx shuffle, packed [128, 8*128].
    # matmul r: out_r[p, :] = k_obj[16r + p%16, :]
    sel = np.zeros((128, 8 * 128), dtype=np.float32)
    for r in range(8):
        for p in range(128):
            sel[16 * r + (p % 16), r * 128 + p] = 1.0
    consts["c_sel"] = sel
    consts["c_iota5"] = np.tile(np.arange(5, dtype=np.float32), (128, 1))
    consts["c_iota5m"] = np.tile(np.arange(5, dtype=np.float32) - 99.0, (128, 1))
    consts["c_iota20"] = np.tile(np.arange(NCLS, dtype=np.float32), (128, 1))
    consts["c_s2"] = np.tile((ANCHORS[0::2] / GRID).astype(np.float32), (128, 1))
    consts["c_s3"] = np.tile((ANCHORS[1::2] / GRID).astype(np.float32), (128, 1))
    # strict upper-triangular pair mask over (o, o2): 1.0 iff o2 > o
    tri = (np.arange(O)[None, :] > np.arange(O)[:, None]).astype(np.float32)
    consts["c_tri"] = np.tile(tri.reshape(1, O * O), (128, 1))
    # imgbase[p, c*64+s] = 169 * (s // 2)  (img_local of wrapped idx slot)
    ib = np.zeros((128, NCHUNK * 64), dtype=np.float32)
    for s in range(64):
        ib[:, np.arange(NCHUNK) * 64 + s] = float(CELLS * (s // 2))
    consts["c_imgbase"] = ib
    return consts


def _dram_ap(t, entries, offset):
    """Build a raw strided AP view over a flat dram tensor."""
    a = t[:].copy()
    a.ap.clear()
    for e in entries:
        a.ap.append(list(e))
    a.offset = offset
    return a


def _build():
    """Build the Bass module (emitted once, cached)."""
    import concourse.bacc as bacc
    import concourse.tile as tile
    from concourse import mybir

    f32 = mybir.dt.float32
    i16 = mybir.dt.int16
    ALU = mybir.AluOpType
    AX = mybir.AxisListType
    ACT = mybir.ActivationFunctionType

    nc = bacc.Bacc(None, target_bir_lowering=False, debug=False)

    det = nc.dram_tensor("det", [BLOC * IMFLT + DPAD], f32,
                         kind="ExternalInput")
    gtb = nc.dram_tensor("gtb", [BLOC, O, 4], f32, kind="ExternalInput")
    clsf = nc.dram_tensor("clsf", [BLOC, O], f32, kind="ExternalInput")
    c_ident = nc.dram_tensor("c_ident", [128, 128], f32, kind="ExternalInput")
    c_sel = nc.dram_tensor("c_sel", [128, 8 * 128], f32, kind="ExternalInput")
    c_iota5 = nc.dram_tensor("c_iota5", [128, 5], f32, kind="ExternalInput")
    c_iota5m = nc.dram_tensor("c_iota5m", [128, 5], f32, kind="ExternalInput")
    c_iota20 = nc.dram_tensor("c_iota20", [128, NCLS], f32, kind="ExternalInput")
    c_s2 = nc.dram_tensor("c_s2", [128, 5], f32, kind="ExternalInput")
    c_s3 = nc.dram_tensor("c_s3", [128, 5], f32, kind="ExternalInput")
    c_tri = nc.dram_tensor("c_tri", [128, O * O], f32, kind="ExternalInput")
    c_imgbase = nc.dram_tensor("c_imgbase", [128, NCHUNK * 64], f32,
                               kind="ExternalInput")
    out = nc.dram_tensor("out", [128, 16], f32, kind="ExternalOutput")

    with tile.TileContext(nc) as tc:
        with tc.tile_pool(name="cpool", bufs=1) as cp, \
             tc.tile_pool(name="work", bufs=1) as wk, \
             tc.tile_pool(name="psA", bufs=2, space="PSUM") as psA, \
             tc.tile_pool(name="psB", bufs=3, space="PSUM") as psB:

            # ---- constants into SBUF (scalar queue; small) ----
            t_id = cp.tile([128, 128], f32)
            t_sel = cp.tile([128, 8 * 128], f32)
            t_i5 = cp.tile([128, 5], f32)
            t_i5m = cp.tile([128, 5], f32)
            t_i20 = cp.tile([128, NCLS], f32)
            t_s2c = cp.tile([128, 5], f32)
            t_s3c = cp.tile([128, 5], f32)
            t_tri = cp.tile([128, O * O], f32)
            t_ib = cp.tile([128, NCHUNK * 64], f32)
            t_cf5 = wk.tile([128, NA * 2 * CELLS], f32)

            # gt / cls bulk loads first: they gate the gather-index chain
            t_gbulk = wk.tile([J2, 512], f32)    # p=j2, f=(bi, o, c)
            nc.scalar.dma_start(
                t_gbulk[:], gtb[:].rearrange("(j bi) o c -> j (bi o c)", bi=4))
            t_cbulk = wk.tile([J2, 128], f32)    # p=j2, f=(bi, o)
            nc.scalar.dma_start(
                t_cbulk[:], clsf[:].rearrange("(j bi) o -> j (bi o)", bi=4))
            nc.scalar.dma_start(t_id[:], c_ident[:])
            nc.scalar.dma_start(t_sel[:], c_sel[:])
            nc.scalar.dma_start(t_ib[:], c_imgbase[:])

            # ---- det chunk loads: 128-partition overlapping window ----
            t_T0 = wk.tile([128, NE], f32)
            t_T1 = wk.tile([128, NE], f32)

            def load_chunk(c):
                t_T = t_T0 if (c % 2 == 0) else t_T1
                src = _dram_ap(
                    det,
                    [[CELLS, 128], [IMFLT, C_IMG], [1, CELLS]],
                    c * C_IMG * IMFLT)
                nc.sync.dma_start(
                    t_T[:].rearrange("p (i e) -> p i e", e=CELLS), src)

            load_chunk(0)
            load_chunk(1)

            # late-use constants (pass math / dedup), off the critical path
            nc.scalar.dma_start(t_i5[:], c_iota5[:])
            nc.scalar.dma_start(t_i5m[:], c_iota5m[:])
            nc.scalar.dma_start(t_i20[:], c_iota20[:])
            nc.scalar.dma_start(t_s2c[:], c_s2[:])
            nc.scalar.dma_start(t_s3c[:], c_s3[:])
            nc.scalar.dma_start(t_tri[:], c_tri[:])

            # ---- gt shuffle to object-major: p=(b%4)*32+o, j2=b//4 ----
            t_gre = wk.tile([J2, 512], f32)      # p=j2, f=(c, bi, o)
            nc.scalar.activation(
                t_gre[:].rearrange("p (c bi o) -> p c bi o", bi=4, o=O),
                t_gbulk[:].rearrange("p (bi o c) -> p c bi o", o=O, c=4),
                ACT.Copy)
            t_gtb = wk.tile([128, J2 * 4], f32)
            gv = t_gtb[:].rearrange("p (j c) -> p j c", c=4)
            for c4 in range(4):
                t_tpg = psA.tile([128, J2], f32, space="PSUM", tag="shuf")
                nc.tensor.transpose(
                    out=t_tpg[:], in_=t_gre[:, c4 * 128:(c4 + 1) * 128],
                    identity=t_id[0:J2, 0:J2])
                nc.scalar.activation(gv[:, :, c4], t_tpg[:], ACT.Copy)
            t_cls = wk.tile([128, J2], f32)
            t_tpc = psA.tile([128, J2], f32, space="PSUM", tag="shuf")
            nc.tensor.transpose(out=t_tpc[:], in_=t_cbulk[:],
                                identity=t_id[0:J2, 0:J2])
            nc.scalar.activation(t_cls[:], t_tpc[:], ACT.Copy)

            x_ap = gv[:, :, 0]
            y_ap = gv[:, :, 1]
            w_ap = gv[:, :, 2]
            h_ap = gv[:, :, 3]

            # ---- cell coords (DVE, object-major [128, 64]) ----
            t_mx = wk.tile([128, J2], f32)
            t_my = wk.tile([128, J2], f32)
            t_tx = wk.tile([128, J2], f32)
            t_ty = wk.tile([128, J2], f32)
            t_gx = wk.tile([128, J2], f32)
            t_gy = wk.tile([128, J2], f32)
            t_k = wk.tile([128, J2], f32)
            t_scr0 = wk.tile([128, J2], f32)
            nc.vector.tensor_scalar_mul(t_mx[:], x_ap, float(GRID))
            nc.vector.tensor_scalar_mul(t_my[:], y_ap, float(GRID))
            # floor(v), robust to the fp->int rounding mode:
            #   i = cvt(v); fi = cvt_back(i); gx = fi - (fi > v)
            t_i32 = wk.tile([128, J2], mybir.dt.int32)
            for t_m_, t_g_ in ((t_mx, t_gx), (t_my, t_gy)):
                nc.vector.tensor_copy(t_i32[:], t_m_[:])
                nc.vector.tensor_copy(t_g_[:], t_i32[:])
                nc.vector.tensor_tensor(t_scr0[:], t_g_[:], t_m_[:], ALU.is_gt)
                nc.vector.tensor_sub(t_g_[:], t_g_[:], t_scr0[:])
            nc.vector.tensor_sub(t_tx[:], t_mx[:], t_gx[:])
            nc.vector.tensor_sub(t_ty[:], t_my[:], t_gy[:])
            nc.vector.scalar_tensor_tensor(
                out=t_k[:], in0=t_gy[:], scalar=float(GRID), in1=t_gx[:],
                op0=ALU.mult, op1=ALU.add)

            # ---- gather-index shuffle into ap_gather's wrapped layout ----
            # idx16[p, c*64+s] = img(s)*169 + k[obj n = c*1024 + 16s + p%16]
            t_idxf = wk.tile([128, NCHUNK * 64], f32)
            for r in range(8):
                t_pr = psA.tile([128, J2], f32, space="PSUM", tag="shuf")
                nc.tensor.matmul(
                    out=t_pr[:], lhsT=t_sel[:, r * 128:(r + 1) * 128],
                    rhs=t_k[:], start=True, stop=True)
                nc.scalar.activation(
                    t_idxf[:].rearrange("p (c sd r) -> p c sd r", sd=8, r=8)
                    [:, :, :, r],
                    t_pr[:].rearrange("p (c sd) -> p c sd", sd=8),
                    ACT.Copy)
            t_idx16 = wk.tile([128, NCHUNK * 64], i16)
            nc.vector.tensor_add(t_idxf[:], t_idxf[:], t_ib[:])
            nc.vector.tensor_copy(t_idx16[:], t_idxf[:])

            # ---- persistent big tiles ----
            t_G0 = wk.tile([128, NIDX], f32)
            t_G1 = wk.tile([128, NIDX], f32)
            t_GTa = wk.tile([128, JPP * NCH], f32)
            t_GTb = wk.tile([128, JPP * NCH], f32)

            t_stage = wk.tile([128, 16], f32)
            nc.vector.memset(t_stage[:], 0.0)

            # per-pass work tiles (reused across passes)
            def w5(nm):
                return wk.tile([128, JPP * NA], f32, name=nm)
            t_iou = w5("t_iou"); t_scr = w5("t_scr"); t_scr2 = w5("t_scr2")
            t_pw = w5("t_pw"); t_ph = w5("t_ph")
            t_bx0 = w5("t_bx0"); t_by0 = w5("t_by0")
            t_bx1 = w5("t_bx1"); t_by1 = w5("t_by1")
            t_ix0 = w5("t_ix0"); t_iy0 = w5("t_iy0")
            t_inter = w5("t_inter"); t_den = w5("t_den")
            t_ohA = w5("t_ohA"); t_csse = w5("t_csse"); t_c1 = w5("t_c1")

            def w1(nm):
                return wk.tile([128, JPP], f32, name=nm)
            t_hw2 = w1("t_hw2"); t_hh2 = w1("t_hh2")
            t_gx0 = w1("t_gx0"); t_gy0 = w1("t_gy0")
            t_gx1 = w1("t_gx1"); t_gy1 = w1("t_gy1")
            t_a1 = w1("t_a1"); t_mm = w1("t_mm")
            t_aidx = w1("t_aidx")
            t_diff = wk.tile([128, JPP * NA * 4], f32)
            t_qcl = wk.tile([128, JPP * NA * NCLS], f32)
            t_oh = wk.tile([128, JPP * NCLS], f32)

            # full-batch tiles (deferred dedup + accumulation)
            t_sid64 = wk.tile([128, J2], f32)
            t_win = wk.tile([128, J2], f32)
            t_terms = wk.tile([128, 4 * J2], f32)   # [p, term, j]
            t_sT = wk.tile([J2, 128], f32)
            t_eqp = wk.tile([J2, 2 * O * O], f32)
            t_dead = wk.tile([J2, 128], f32)

            def r5(t, sl=slice(None)):
                return t[:].rearrange("p (j a) -> p j a", a=NA)[:, sl]

            tv = t_terms[:].rearrange("p (t j) -> p t j", j=J2)

            def gather_chunk(c):
                t_T = t_T0 if (c % 2 == 0) else t_T1
                t_G = t_G0 if (c % 2 == 0) else t_G1
                nc.gpsimd.ap_gather(
                    out_ap=t_G[:],
                    in_ap=t_T[:],
                    idxs_ap=t_idx16[:, c * 64:(c + 1) * 64],
                    channels=128, num_elems=NE, d=1, num_idxs=NIDX)

            def tr_copies(c):
                t_G = t_G0 if (c % 2 == 0) else t_G1
                t_GT = t_GTa if ((c // CPP) % 2 == 0) else t_GTb
                for j in range(NIDX // 128):
                    t_tp = psB.tile([128, 128], f32, space="PSUM", tag="tp")
                    nc.tensor.transpose(
                        out=t_tp[:], in_=t_G[:, j * 128:(j + 1) * 128],
                        identity=t_id[:])
                    jg = (c % CPP) * 8 + j
                    nc.scalar.activation(
                        t_GT[:, jg * NCH:(jg + 1) * NCH],
                        t_tp[:, 0:NCH], ACT.Copy)

            def pass_math(ps):
                t_GT = t_GTa if (ps % 2 == 0) else t_GTb
                jsl = slice(ps * JPP, (ps + 1) * JPP)
                gtv = t_GT[:].rearrange("p (j a r) -> p j a r",
                                        a=NA, r=CH)
                q0 = gtv[:, :, :, 0]
                q1 = gtv[:, :, :, 1]
                q2 = gtv[:, :, :, 2]
                q3 = gtv[:, :, :, 3]
                q4 = gtv[:, :, :, 4]
                qclsv = gtv[:, :, :, 5:CH]          # [p, j, a, 20]

                def b5(ap2d):  # [128, JPP] -> broadcast [128, JPP, 5]
                    return ap2d.rearrange("p (j one) -> p j one", one=1) \
                               .to_broadcast([128, JPP, NA])

                def c5(tile1):  # const [128, 5] -> [128, JPP, 5]
                    return tile1[:].rearrange("p (one a) -> p one a", one=1) \
                                   .to_broadcast([128, JPP, NA])

                # ---- IoU (per object x anchor) ----
                nc.vector.tensor_tensor(r5(t_pw), q2, c5(t_s2c), ALU.mult)
                nc.vector.tensor_tensor(r5(t_ph), q3, c5(t_s3c), ALU.mult)
                # bx0 = (px+gx)/13 - pw/2 ; by0 = (py+gy)/13 - ph/2
                nc.vector.tensor_tensor(r5(t_bx0), q0, b5(t_gx[:, jsl]), ALU.add)
                nc.vector.tensor_scalar_mul(t_bx0[:], t_bx0[:], 1.0 / GRID)
                nc.vector.scalar_tensor_tensor(
                    out=t_bx0[:], in0=t_pw[:], scalar=-0.5, in1=t_bx0[:],
                    op0=ALU.mult, op1=ALU.add)
                nc.vector.tensor_tensor(r5(t_by0), q1, b5(t_gy[:, jsl]), ALU.add)
                nc.vector.tensor_scalar_mul(t_by0[:], t_by0[:], 1.0 / GRID)
                nc.vector.scalar_tensor_tensor(
                    out=t_by0[:], in0=t_ph[:], scalar=-0.5, in1=t_by0[:],
                    op0=ALU.mult, op1=ALU.add)
                nc.vector.tensor_add(t_bx1[:], t_bx0[:], t_pw[:])
                nc.vector.tensor_add(t_by1[:], t_by0[:], t_ph[:])
                # gt box corners [128, JPP]
                nc.vector.tensor_scalar_mul(t_hw2[:], w_ap[:, jsl], 0.5)
                nc.vector.tensor_scalar_mul(t_hh2[:], h_ap[:, jsl], 0.5)
                nc.vector.tensor_sub(t_gx0[:], x_ap[:, jsl], t_hw2[:])
                nc.vector.tensor_add(t_gx1[:], x_ap[:, jsl], t_hw2[:])
                nc.vector.tensor_sub(t_gy0[:], y_ap[:, jsl], t_hh2[:])
                nc.vector.tensor_add(t_gy1[:], y_ap[:, jsl], t_hh2[:])
                # a1 = (gx1-gx0+1)*(gy1-gy0+1)
                nc.vector.tensor_sub(t_a1[:], t_gx1[:], t_gx0[:])
                nc.vector.tensor_scalar_add(t_a1[:], t_a1[:], 1.0)
                nc.vector.tensor_sub(t_mm[:], t_gy1[:], t_gy0[:])
                nc.vector.tensor_scalar_add(t_mm[:], t_mm[:], 1.0)
                nc.vector.tensor_mul(t_a1[:], t_a1[:], t_mm[:])
                # intersection: ix0 = max(gx0,bx0); ix1 = min(gx1,bx1) (in bx1)
                nc.vector.tensor_tensor(r5(t_ix0), r5(t_bx0), b5(t_gx0), ALU.max)
                nc.vector.tensor_tensor(r5(t_iy0), r5(t_by0), b5(t_gy0), ALU.max)
                nc.vector.tensor_tensor(r5(t_bx1), r5(t_bx1), b5(t_gx1), ALU.min)
                nc.vector.tensor_tensor(r5(t_by1), r5(t_by1), b5(t_gy1), ALU.min)
                nc.vector.tensor_sub(t_bx1[:], t_bx1[:], t_ix0[:])
                nc.vector.tensor_scalar_add(t_bx1[:], t_bx1[:], 1.0)
                nc.vector.tensor_sub(t_by1[:], t_by1[:], t_iy0[:])
                nc.vector.tensor_scalar_add(t_by1[:], t_by1[:], 1.0)
                nc.vector.tensor_mul(t_inter[:], t_bx1[:], t_by1[:])
                # a2 = (pw+1)*(ph+1); denom = a1 + a2 - inter
                nc.vector.tensor_scalar_add(t_pw[:], t_pw[:], 1.0)
                nc.vector.tensor_scalar_add(t_ph[:], t_ph[:], 1.0)
                nc.vector.tensor_mul(t_den[:], t_pw[:], t_ph[:])
                nc.vector.tensor_tensor(r5(t_den), r5(t_den), b5(t_a1), ALU.add)
                nc.vector.tensor_sub(t_den[:], t_den[:], t_inter[:])
                nc.vector.reciprocal(t_den[:], t_den[:])
                nc.vector.tensor_mul(t_iou[:], t_inter[:], t_den[:])

                # ---- argmax over anchors (first max wins) ----
                nc.vector.reduce_max(t_mm[:], r5(t_iou), axis=AX.X)
                nc.vector.tensor_tensor(
                    r5(t_scr), r5(t_iou), b5(t_mm), ALU.is_equal)
                nc.vector.tensor_tensor(
                    r5(t_scr2), r5(t_scr), c5(t_i5m), ALU.mult)
                nc.vector.tensor_reduce(
                    t_aidx[:], r5(t_scr2), axis=AX.X, op=ALU.min)
                nc.vector.tensor_scalar_add(t_aidx[:], t_aidx[:], 99.0)

                # ---- slot id s = 169*aidx + k (dedup deferred) ----
                nc.vector.scalar_tensor_tensor(
                    out=t_sid64[:, jsl], in0=t_aidx[:], scalar=float(CELLS),
                    in1=t_k[:, jsl], op0=ALU.mult, op1=ALU.add)

                # ---- anchor one-hot ----
                nc.vector.tensor_tensor(
                    r5(t_ohA), b5(t_aidx), c5(t_i5), ALU.is_equal)

                # ---- coord SSE, winner-selected ----
                dv = t_diff[:].rearrange("p (j a c) -> p j a c", a=NA, c=4)
                nc.vector.tensor_tensor(
                    dv[:, :, :, 0], q0, b5(t_tx[:, jsl]), ALU.subtract)
                nc.vector.tensor_tensor(
                    dv[:, :, :, 1], q1, b5(t_ty[:, jsl]), ALU.subtract)
                nc.vector.tensor_tensor(r5(t_scr), q2, c5(t_s2c), ALU.mult)
                nc.vector.tensor_tensor(
                    dv[:, :, :, 2], r5(t_scr), b5(w_ap[:, jsl]), ALU.subtract)
                nc.vector.tensor_tensor(r5(t_scr), q3, c5(t_s3c), ALU.mult)
                nc.vector.tensor_tensor(
                    dv[:, :, :, 3], r5(t_scr), b5(h_ap[:, jsl]), ALU.subtract)
                nc.vector.tensor_mul(t_diff[:], t_diff[:], t_diff[:])
                nc.vector.tensor_reduce(r5(t_csse), dv, axis=AX.X, op=ALU.add)
                nc.vector.tensor_mul(t_csse[:], t_csse[:], t_ohA[:])
                nc.vector.tensor_reduce(
                    tv[:, 0, jsl], r5(t_csse), axis=AX.X, op=ALU.add)

                # ---- conf terms at slots: (1-q4)^2 and q4^2, selected ----
                nc.vector.tensor_scalar(
                    r5(t_c1), q4, -1.0, 1.0, ALU.mult, ALU.add)
                nc.vector.tensor_mul(t_c1[:], t_c1[:], t_c1[:])
                nc.vector.tensor_mul(t_c1[:], t_c1[:], t_ohA[:])
                nc.vector.tensor_reduce(
                    tv[:, 1, jsl], r5(t_c1), axis=AX.X, op=ALU.add)
                nc.vector.tensor_tensor(r5(t_scr), q4, q4, ALU.mult)
                nc.vector.tensor_mul(t_scr[:], t_scr[:], t_ohA[:])
                nc.vector.tensor_reduce(
                    tv[:, 2, jsl], r5(t_scr), axis=AX.X, op=ALU.add)

                # ---- class term: sel_a sum_cls q*(q - 2*onehot20) ----
                ohv = t_oh[:].rearrange("p (j c) -> p j c", c=NCLS)
                nc.vector.tensor_tensor(
                    ohv,
                    t_cls[:, jsl].rearrange("p (j one) -> p j one", one=1)
                    .to_broadcast([128, JPP, NCLS]),
                    t_i20[:].rearrange("p (one c) -> p one c", one=1)
                    .to_broadcast([128, JPP, NCLS]),
                    ALU.is_equal)
                qcv = t_qcl[:].rearrange("p (j a c) -> p j a c", a=NA, c=NCLS)
                for a in range(NA):
                    nc.vector.scalar_tensor_tensor(
                        out=qcv[:, :, a, :], in0=ohv, scalar=-2.0,
                        in1=qclsv[:, :, a, :], op0=ALU.mult, op1=ALU.add)
                nc.vector.tensor_tensor(qcv, qcv, qclsv, ALU.mult)
                nc.vector.tensor_reduce(r5(t_scr), qcv, axis=AX.X, op=ALU.add)
                nc.vector.tensor_mul(t_scr[:], t_scr[:], t_ohA[:])
                nc.vector.tensor_reduce(
                    tv[:, 3, jsl], r5(t_scr), axis=AX.X, op=ALU.add)

            # ---- chunk pipeline ----
            for c in range(NCHUNK):
                gather_chunk(c)
                if c + 2 < NCHUNK:
                    load_chunk(c + 2)
                tr_copies(c)
                if c % 2 == 1:
                    pass_math(c // 2)

            # ---- dense conf channels: det[:, 25a+4, :] as [128, 2*169] ----
            for a in range(NA):
                src = _dram_ap(
                    det,
                    [[IMFLT, 128], [128 * IMFLT, 2], [1, CELLS]],
                    (a * CH + 4) * CELLS)
                nc.scalar.dma_start(
                    t_cf5[:, a * 2 * CELLS:(a + 1) * 2 * CELLS]
                    .rearrange("p (bh e) -> p bh e", e=CELLS), src)

            # ---- deferred last-writer-wins dedup over (aidx, cell) ----
            t_tps = psA.tile([J2, 128], f32, space="PSUM", tag="ded", bufs=1)
            nc.tensor.transpose(out=t_tps[:], in_=t_sid64[:], identity=t_id[:])
            nc.scalar.activation(t_sT[:], t_tps[:], ACT.Copy)
            for h in range(2):
                sl128 = slice(h * 64, (h + 1) * 64)
                sTa = t_sT[:, sl128].rearrange(
                    "p (bi o one) -> p bi o one", bi=2, one=1) \
                    .to_broadcast([J2, 2, O, O])
                sTb = t_sT[:, sl128].rearrange(
                    "p (bi one o2) -> p bi one o2", bi=2, one=1) \
                    .to_broadcast([J2, 2, O, O])
                eqv = t_eqp[:].rearrange("p (bi o o2) -> p bi o o2",
                                         bi=2, o2=O)
                nc.vector.tensor_tensor(eqv, sTa, sTb, ALU.is_equal)
                triv = t_tri[0:J2, :].rearrange(
                    "p (one o o2) -> p one o o2", one=1, o2=O) \
                    .to_broadcast([J2, 2, O, O])
                nc.vector.tensor_tensor(eqv, eqv, triv, ALU.mult)
                nc.vector.tensor_reduce(
                    t_dead[:, sl128].rearrange("p (bi o) -> p bi o", o=O),
                    eqv, axis=AX.X, op=ALU.max)
            t_tpw = psA.tile([128, J2], f32, space="PSUM", tag="ded2", bufs=1)
            nc.tensor.transpose(
                out=t_tpw[:], in_=t_dead[:], identity=t_id[0:J2, 0:J2])
            nc.scalar.activation(t_win[:], t_tpw[:], ACT.Copy)
            nc.vector.tensor_scalar(
                t_win[:], t_win[:], -1.0, 1.0, ALU.mult, ALU.add)

            # ---- masked accumulation into staging ----
            nc.vector.tensor_tensor(
                tv, tv,
                t_win[:].rearrange("p (one j) -> p one j", one=1)
                .to_broadcast([128, 4, J2]),
                ALU.mult)
            nc.vector.tensor_reduce(
                t_stage[:, 0:4].rearrange("p (t one) -> p t one", one=1),
                tv, axis=AX.X, op=ALU.add)
            nc.vector.reduce_sum(t_stage[:, 4:5], t_win[:], axis=AX.X)

            # ---- dense conf sum: square + reduce per anchor ----
            nc.vector.tensor_mul(t_cf5[:], t_cf5[:], t_cf5[:])
            nc.vector.tensor_reduce(
                t_stage[:, 5:10].rearrange("p (a one) -> p a one", one=1),
                t_cf5[:].rearrange("p (a e) -> p a e", a=NA),
                axis=AX.X, op=ALU.add)

            nc.sync.dma_start(out[:], t_stage[:])

    nc.compile()
    return nc


def _get_built():
    if "nc" not in _CACHE:
        _CACHE["nc"] = _build()
        _CACHE["consts"] = _make_consts()
    return _CACHE["nc"], _CACHE["consts"]


def _reduce_partials(P):
    """P: [ncores, 128, 16] fp32 partials -> the 4 scalar losses."""
    S = P.astype(np.float64).sum(axis=(0, 1))
    coord, confobj, confsub, clsq, wsum = S[0], S[1], S[2], S[3], S[4]
    dense = S[5:10].sum()
    obj_loss = 5.0 * coord + confobj
    no_obj_loss = 0.5 * (dense - confsub)
    conf_loss = clsq + wsum
    loss = obj_loss + no_obj_loss + conf_loss
    return (np.float32(loss), np.float32(obj_loss),
            np.float32(no_obj_loss), np.float32(conf_loss))


def _make_in_maps(detection_result, gt_boxes, gt_class, consts):
    det = np.ascontiguousarray(
        np.asarray(detection_result, dtype=np.float32)).reshape(B, -1)
    gtb = np.ascontiguousarray(np.asarray(gt_boxes, dtype=np.float32))
    clsf = np.asarray(gt_class).astype(np.float32)

    pad = np.zeros(DPAD, dtype=np.float32)
    in_maps = []
    for c in range(NCORES):
        sl = slice(c * BLOC, (c + 1) * BLOC)
        m = {"det": np.concatenate([det[sl].ravel(), pad]),
             "gtb": gtb[sl], "clsf": clsf[sl]}
        m.update(consts)
        in_maps.append(m)
    return in_maps


def kernel(detection_result, gt_boxes, gt_class):
    from concourse.bass_utils import run_bass_kernel_spmd

    nc, consts = _get_built()
    in_maps = _make_in_maps(detection_result, gt_boxes, gt_class, consts)
    res = run_bass_kernel_spmd(nc, in_maps, core_ids=list(range(NCORES)))
    P = np.stack([res.results[c]["out"] for c in range(NCORES)])
    return _reduce_partials(P)


# revision 29
# speedup vs baseline: 1.0188x; 1.0188x over previous
"""Trainium2 Bass kernel for nn_DetectionLoss (YOLO-style detection loss).

Strategy (pure data parallel over 8 NeuronCores, 256 images each):
  - det loaded channel-major in 32-img chunks with a 128-partition
    overlapping-window AP (partition p reads channel p of each image, with
    p in [125,128) spilling into the next image's channels; the host pads
    the flat det buffer so the final image stays in bounds). 128-partition
    DMAs spread evenly over all 16 SDMA engines (~286 GB/s vs ~99 GB/s for
    the 125-partition layout).
  - One GPSIMD ap_gather per chunk pulls all 128 channel-partitions at the
    32 object cells per image -> G[128ch, obj].
  - PE transposes G into object-major GT[128obj, ch] tiles (ACT copies).
  - DVE does IoU / argmax / per-object loss terms in 2 pipelined passes;
    last-writer-wins dedup is deferred to a single [64-partition, 2048]
    pair-compare at the end.
  - Dense no-obj conf sum via a separate per-anchor reload of channel 4.
  - Output: per-core partial sums [128, 16]; host reduces across cores.
"""
import numpy as np

GRID = 13
NA = 5
NCLS = 20
CH = 25
NCH = NA * CH          # 125
CELLS = GRID * GRID    # 169
O = 32                 # objects per image
B = 2048               # global batch
NCORES = 8
BLOC = B // NCORES     # 256 images per core
C_IMG = 32             # images per chunk
NCHUNK = BLOC // C_IMG                   # 8
NE = C_IMG * CELLS                       # 5408 elems/partition per chunk
NIDX = C_IMG * O                         # 1024 gather idxs per chunk
NOBJ = BLOC * O                          # 8192 objects per core
J2 = NOBJ // 128                         # 64 object columns
IMFLT = NCH * CELLS                      # 21125 floats per image
DPAD = 1024                              # host-side pad floats on det
NPASS = 4
CPP = NCHUNK // NPASS                    # chunks per pass (2)
JPP = J2 // NPASS                        # j2 per pass (16)

ANCHORS = np.array([1.3221, 1.73145, 3.19275, 4.00944, 5.05587,
                    8.09892, 9.47112, 4.84053, 11.2364, 10.0071],
                   dtype=np.float32)

_CACHE = {}


def _make_consts():
    """Host-precomputed, data-independent constant input tensors."""
    consts = {}
    consts["c_ident"] = np.eye(128, dtype=np.float32)
    # 8 partition-selector matrices for the idx shuffle, packed [128, 8*128].
    # matmul r: out_r[p, :] = k_obj[16r + p%16, :]
    sel = np.zeros((128, 8 * 128), dtype=np.float32)
    for r in range(8):
        for p in range(128):
            sel[16 * r + (p % 16), r * 128 + p] = 1.0
    consts["c_sel"] = sel
    consts["c_iota5"] = np.tile(np.arange(5, dtype=np.float32), (128, 1))
    consts["c_iota5m"] = np.tile(np.arange(5, dtype=np.float32) - 99.0, (128, 1))
    consts["c_iota20"] = np.tile(np.arange(NCLS, dtype=np.float32), (128, 1))
    consts["c_s2"] = np.tile((ANCHORS[0::2] / GRID).astype(np.float32), (128, 1))
    consts["c_s3"] = np.tile((ANCHORS[1::2] / GRID).astype(np.float32), (128, 1))
    # strict upper-triangular pair mask over (o, o2): 1.0 iff o2 > o
    tri = (np.arange(O)[None, :] > np.arange(O)[:, None]).astype(np.float32)
    consts["c_tri"] = np.tile(tri.reshape(1, O * O), (128, 1))
    # imgbase[p, c*64+s] = 169 * (s // 2)  (img_local of wrapped idx slot)
    ib = np.zeros((128, NCHUNK * 64), dtype=np.float32)
    for s in range(64):
        ib[:, np.arange(NCHUNK) * 64 + s] = float(CELLS * (s // 2))
    consts["c_imgbase"] = ib
    return consts


def _dram_ap(t, entries, offset):
    """Build a raw strided AP view over a flat dram tensor."""
    a = t[:].copy()
    a.ap.clear()
    for e in entries:
        a.ap.append(list(e))
    a.offset = offset
    return a


def _build():
    """Build the Bass module (emitted once, cached)."""
    import concourse.bacc as bacc
    import concourse.tile as tile
    from concourse import mybir

    f32 = mybir.dt.float32
    i16 = mybir.dt.int16
    ALU = mybir.AluOpType
    AX = mybir.AxisListType
    ACT = mybir.ActivationFunctionType

    nc = bacc.Bacc(None, target_bir_lowering=False, debug=False)

    det = nc.dram_tensor("det", [BLOC * IMFLT + DPAD], f32,
                         kind="ExternalInput")
    gtb = nc.dram_tensor("gtb", [BLOC, O, 4], f32, kind="ExternalInput")
    clsf = nc.dram_tensor("clsf", [BLOC, O], f32, kind="ExternalInput")
    c_ident = nc.dram_tensor("c_ident", [128, 128], f32, kind="ExternalInput")
    c_sel = nc.dram_tensor("c_sel", [128, 8 * 128], f32, kind="ExternalInput")
    c_iota5 = nc.dram_tensor("c_iota5", [128, 5], f32, kind="ExternalInput")
    c_iota5m = nc.dram_tensor("c_iota5m", [128, 5], f32, kind="ExternalInput")
    c_iota20 = nc.dram_tensor("c_iota20", [128, NCLS], f32, kind="ExternalInput")
    c_s2 = nc.dram_tensor("c_s2", [128, 5], f32, kind="ExternalInput")
    c_s3 = nc.dram_tensor("c_s3", [128, 5], f32, kind="ExternalInput")
    c_tri = nc.dram_tensor("c_tri", [128, O * O], f32, kind="ExternalInput")
    c_imgbase = nc.dram_tensor("c_imgbase", [128, NCHUNK * 64], f32,
                               kind="ExternalInput")
    out = nc.dram_tensor("out", [128, 16], f32, kind="ExternalOutput")

    with tile.TileContext(nc) as tc:
        with tc.tile_pool(name="cpool", bufs=1) as cp, \
             tc.tile_pool(name="work", bufs=1) as wk, \
             tc.tile_pool(name="psA", bufs=2, space="PSUM") as psA, \
             tc.tile_pool(name="psB", bufs=3, space="PSUM") as psB:

            # ---- constants into SBUF (scalar queue; small) ----
            t_id = cp.tile([128, 128], f32)
            t_sel = cp.tile([128, 8 * 128], f32)
            t_i5 = cp.tile([128, 5], f32)
            t_i5m = cp.tile([128, 5], f32)
            t_i20 = cp.tile([128, NCLS], f32)
            t_s2c = cp.tile([128, 5], f32)
            t_s3c = cp.tile([128, 5], f32)
            t_tri = cp.tile([128, O * O], f32)
            t_ib = cp.tile([128, NCHUNK * 64], f32)
            t_cf5 = wk.tile([128, NA * 2 * CELLS], f32)

            # gt / cls bulk loads first: they gate the gather-index chain
            t_gbulk = wk.tile([J2, 512], f32)    # p=j2, f=(bi, o, c)
            nc.scalar.dma_start(
                t_gbulk[:], gtb[:].rearrange("(j bi) o c -> j (bi o c)", bi=4))
            t_cbulk = wk.tile([J2, 128], f32)    # p=j2, f=(bi, o)
            nc.scalar.dma_start(
                t_cbulk[:], clsf[:].rearrange("(j bi) o -> j (bi o)", bi=4))
            nc.scalar.dma_start(t_id[:], c_ident[:])
            nc.scalar.dma_start(t_sel[:], c_sel[:])
            nc.scalar.dma_start(t_ib[:], c_imgbase[:])

            # ---- det chunk loads: 128-partition overlapping window ----
            t_T0 = wk.tile([128, NE], f32)
            t_T1 = wk.tile([128, NE], f32)

            def load_chunk(c):
                t_T = t_T0 if (c % 2 == 0) else t_T1
                src = _dram_ap(
                    det,
                    [[CELLS, 128], [IMFLT, C_IMG], [1, CELLS]],
                    c * C_IMG * IMFLT)
                nc.sync.dma_start(
                    t_T[:].rearrange("p (i e) -> p i e", e=CELLS), src)

            # chunk 0 loads in two halves so the first (half-)gather only
            # waits on 16 images of data
            H = C_IMG // 2
            for h in range(2):
                src = _dram_ap(
                    det,
                    [[CELLS, 128], [IMFLT, H], [1, CELLS]],
                    h * H * IMFLT)
                nc.sync.dma_start(
                    t_T0[:, h * H * CELLS:(h + 1) * H * CELLS]
                    .rearrange("p (i e) -> p i e", e=CELLS), src)
            load_chunk(1)

            # late-use constants (pass math / dedup), off the critical path
            nc.scalar.dma_start(t_i5[:], c_iota5[:])
            nc.scalar.dma_start(t_i5m[:], c_iota5m[:])
            nc.scalar.dma_start(t_i20[:], c_iota20[:])
            nc.scalar.dma_start(t_s2c[:], c_s2[:])
            nc.scalar.dma_start(t_s3c[:], c_s3[:])
            nc.scalar.dma_start(t_tri[:], c_tri[:])

            # ---- gt shuffle to object-major: p=(b%4)*32+o, j2=b//4 ----
            t_gre = wk.tile([J2, 512], f32)      # p=j2, f=(c, bi, o)
            nc.scalar.activation(
                t_gre[:].rearrange("p (c bi o) -> p c bi o", bi=4, o=O),
                t_gbulk[:].rearrange("p (bi o c) -> p c bi o", o=O, c=4),
                ACT.Copy)
            t_gtb = wk.tile([128, J2 * 4], f32)
            gv = t_gtb[:].rearrange("p (j c) -> p j c", c=4)
            for c4 in range(4):
                t_tpg = psA.tile([128, J2], f32, space="PSUM", tag="shuf")
                nc.tensor.transpose(
                    out=t_tpg[:], in_=t_gre[:, c4 * 128:(c4 + 1) * 128],
                    identity=t_id[0:J2, 0:J2])
                nc.scalar.activation(gv[:, :, c4], t_tpg[:], ACT.Copy)
            t_cls = wk.tile([128, J2], f32)
            t_tpc = psA.tile([128, J2], f32, space="PSUM", tag="shuf")
            nc.tensor.transpose(out=t_tpc[:], in_=t_cbulk[:],
                                identity=t_id[0:J2, 0:J2])
            nc.scalar.activation(t_cls[:], t_tpc[:], ACT.Copy)

            x_ap = gv[:, :, 0]
            y_ap = gv[:, :, 1]
            w_ap = gv[:, :, 2]
            h_ap = gv[:, :, 3]

            # ---- cell coords (DVE, object-major [128, 64]) ----
            t_mx = wk.tile([128, J2], f32)
            t_my = wk.tile([128, J2], f32)
            t_tx = wk.tile([128, J2], f32)
            t_ty = wk.tile([128, J2], f32)
            t_gx = wk.tile([128, J2], f32)
            t_gy = wk.tile([128, J2], f32)
            t_k = wk.tile([128, J2], f32)
            t_scr0 = wk.tile([128, J2], f32)
            nc.vector.tensor_scalar_mul(t_mx[:], x_ap, float(GRID))
            nc.vector.tensor_scalar_mul(t_my[:], y_ap, float(GRID))
            # floor(v), robust to the fp->int rounding mode:
            #   i = cvt(v); fi = cvt_back(i); gx = fi - (fi > v)
            t_i32 = wk.tile([128, J2], mybir.dt.int32)
            for t_m_, t_g_ in ((t_mx, t_gx), (t_my, t_gy)):
                nc.vector.tensor_copy(t_i32[:], t_m_[:])
                nc.vector.tensor_copy(t_g_[:], t_i32[:])
                nc.vector.tensor_tensor(t_scr0[:], t_g_[:], t_m_[:], ALU.is_gt)
                nc.vector.tensor_sub(t_g_[:], t_g_[:], t_scr0[:])
            nc.vector.tensor_sub(t_tx[:], t_mx[:], t_gx[:])
            nc.vector.tensor_sub(t_ty[:], t_my[:], t_gy[:])
            nc.vector.scalar_tensor_tensor(
                out=t_k[:], in0=t_gy[:], scalar=float(GRID), in1=t_gx[:],
                op0=ALU.mult, op1=ALU.add)

            # ---- gather-index shuffle into ap_gather's wrapped layout ----
            # idx16[p, c*64+s] = img(s)*169 + k[obj n = c*1024 + 16s + p%16]
            t_idxf = wk.tile([128, NCHUNK * 64], f32)
            for r in range(8):
                t_pr = psA.tile([128, J2], f32, space="PSUM", tag="shuf")
                nc.tensor.matmul(
                    out=t_pr[:], lhsT=t_sel[:, r * 128:(r + 1) * 128],
                    rhs=t_k[:], start=True, stop=True)
                nc.scalar.activation(
                    t_idxf[:].rearrange("p (c sd r) -> p c sd r", sd=8, r=8)
                    [:, :, :, r],
                    t_pr[:].rearrange("p (c sd) -> p c sd", sd=8),
                    ACT.Copy)
            t_idx16 = wk.tile([128, NCHUNK * 64], i16)
            nc.vector.tensor_add(t_idxf[:], t_idxf[:], t_ib[:])
            nc.vector.tensor_copy(t_idx16[:], t_idxf[:])

            # ---- persistent big tiles ----
            t_G0 = wk.tile([128, NIDX], f32)
            t_G1 = wk.tile([128, NIDX], f32)
            t_GTa = wk.tile([128, JPP * NCH], f32)
            t_GTb = wk.tile([128, JPP * NCH], f32)

            t_stage = wk.tile([128, 16], f32)
            nc.vector.memset(t_stage[:], 0.0)

            # per-pass work tiles (reused across passes)
            def w5(nm):
                return wk.tile([128, JPP * NA], f32, name=nm)
            t_iou = w5("t_iou"); t_scr = w5("t_scr"); t_scr2 = w5("t_scr2")
            t_pw = w5("t_pw"); t_ph = w5("t_ph")
            t_bx0 = w5("t_bx0"); t_by0 = w5("t_by0")
            t_bx1 = w5("t_bx1"); t_by1 = w5("t_by1")
            t_ix0 = w5("t_ix0"); t_iy0 = w5("t_iy0")
            t_inter = w5("t_inter"); t_den = w5("t_den")
            t_ohA = w5("t_ohA"); t_csse = w5("t_csse"); t_c1 = w5("t_c1")

            def w1(nm):
                return wk.tile([128, JPP], f32, name=nm)
            t_hw2 = w1("t_hw2"); t_hh2 = w1("t_hh2")
            t_gx0 = w1("t_gx0"); t_gy0 = w1("t_gy0")
            t_gx1 = w1("t_gx1"); t_gy1 = w1("t_gy1")
            t_a1 = w1("t_a1"); t_mm = w1("t_mm")
            t_aidx = w1("t_aidx")
            t_diff = wk.tile([128, JPP * NA * 4], f32)
            t_qcl = wk.tile([128, JPP * NA * NCLS], f32)
            t_oh = wk.tile([128, JPP * NCLS], f32)

            # full-batch tiles (deferred dedup + accumulation)
            t_sid64 = wk.tile([128, J2], f32)
            t_win = wk.tile([128, J2], f32)
            t_terms = wk.tile([128, 4 * J2], f32)   # [p, term, j]
            t_sT = wk.tile([J2, 128], f32)
            t_eqp = wk.tile([J2, 2 * O * O], f32)
            t_dead = wk.tile([J2, 128], f32)

            def r5(t, sl=slice(None)):
                return t[:].rearrange("p (j a) -> p j a", a=NA)[:, sl]

            tv = t_terms[:].rearrange("p (t j) -> p t j", j=J2)

            def gather_chunk(c):
                t_T = t_T0 if (c % 2 == 0) else t_T1
                t_G = t_G0 if (c % 2 == 0) else t_G1
                nc.gpsimd.ap_gather(
                    out_ap=t_G[:],
                    in_ap=t_T[:],
                    idxs_ap=t_idx16[:, c * 64:(c + 1) * 64],
                    channels=128, num_elems=NE, d=1, num_idxs=NIDX)

            def tr_copies(c):
                t_G = t_G0 if (c % 2 == 0) else t_G1
                t_GT = t_GTa if ((c // CPP) % 2 == 0) else t_GTb
                for j in range(NIDX // 128):
                    t_tp = psB.tile([128, 128], f32, space="PSUM", tag="tp")
                    nc.tensor.transpose(
                        out=t_tp[:], in_=t_G[:, j * 128:(j + 1) * 128],
                        identity=t_id[:])
                    jg = (c % CPP) * 8 + j
                    nc.scalar.activation(
                        t_GT[:, jg * NCH:(jg + 1) * NCH],
                        t_tp[:, 0:NCH], ACT.Copy)

            def pass_math(ps):
                t_GT = t_GTa if (ps % 2 == 0) else t_GTb
                jsl = slice(ps * JPP, (ps + 1) * JPP)
                gtv = t_GT[:].rearrange("p (j a r) -> p j a r",
                                        a=NA, r=CH)
                q0 = gtv[:, :, :, 0]
                q1 = gtv[:, :, :, 1]
                q2 = gtv[:, :, :, 2]
                q3 = gtv[:, :, :, 3]
                q4 = gtv[:, :, :, 4]
                qclsv = gtv[:, :, :, 5:CH]          # [p, j, a, 20]

                def b5(ap2d):  # [128, JPP] -> broadcast [128, JPP, 5]
                    return ap2d.rearrange("p (j one) -> p j one", one=1) \
                               .to_broadcast([128, JPP, NA])

                def c5(tile1):  # const [128, 5] -> [128, JPP, 5]
                    return tile1[:].rearrange("p (one a) -> p one a", one=1) \
                                   .to_broadcast([128, JPP, NA])

                # ---- IoU (per object x anchor) ----
                nc.vector.tensor_tensor(r5(t_pw), q2, c5(t_s2c), ALU.mult)
                nc.vector.tensor_tensor(r5(t_ph), q3, c5(t_s3c), ALU.mult)
                # bx0 = (px+gx)/13 - pw/2 ; by0 = (py+gy)/13 - ph/2
                nc.vector.tensor_tensor(r5(t_bx0), q0, b5(t_gx[:, jsl]), ALU.add)
                nc.vector.tensor_scalar_mul(t_bx0[:], t_bx0[:], 1.0 / GRID)
                nc.vector.scalar_tensor_tensor(
                    out=t_bx0[:], in0=t_pw[:], scalar=-0.5, in1=t_bx0[:],
                    op0=ALU.mult, op1=ALU.add)
                nc.vector.tensor_tensor(r5(t_by0), q1, b5(t_gy[:, jsl]), ALU.add)
                nc.vector.tensor_scalar_mul(t_by0[:], t_by0[:], 1.0 / GRID)
                nc.vector.scalar_tensor_tensor(
                    out=t_by0[:], in0=t_ph[:], scalar=-0.5, in1=t_by0[:],
                    op0=ALU.mult, op1=ALU.add)
                nc.vector.tensor_add(t_bx1[:], t_bx0[:], t_pw[:])
                nc.vector.tensor_add(t_by1[:], t_by0[:], t_ph[:])
                # gt box corners [128, JPP]
                nc.vector.tensor_scalar_mul(t_hw2[:], w_ap[:, jsl], 0.5)
                nc.vector.tensor_scalar_mul(t_hh2[:], h_ap[:, jsl], 0.5)
                nc.vector.tensor_sub(t_gx0[:], x_ap[:, jsl], t_hw2[:])
                nc.vector.tensor_add(t_gx1[:], x_ap[:, jsl], t_hw2[:])
                nc.vector.tensor_sub(t_gy0[:], y_ap[:, jsl], t_hh2[:])
                nc.vector.tensor_add(t_gy1[:], y_ap[:, jsl], t_hh2[:])
                # a1 = (gx1-gx0+1)*(gy1-gy0+1)
                nc.vector.tensor_sub(t_a1[:], t_gx1[:], t_gx0[:])
                nc.vector.tensor_scalar_add(t_a1[:], t_a1[:], 1.0)
                nc.vector.tensor_sub(t_mm[:], t_gy1[:], t_gy0[:])
                nc.vector.tensor_scalar_add(t_mm[:], t_mm[:], 1.0)
                nc.vector.tensor_mul(t_a1[:], t_a1[:], t_mm[:])
                # intersection: ix0 = max(gx0,bx0); ix1 = min(gx1,bx1) (in bx1)
                nc.vector.tensor_tensor(r5(t_ix0), r5(t_bx0), b5(t_gx0), ALU.max)
                nc.vector.tensor_tensor(r5(t_iy0), r5(t_by0), b5(t_gy0), ALU.max)
                nc.vector.tensor_tensor(r5(t_bx1), r5(t_bx1), b5(t_gx1), ALU.min)
                nc.vector.tensor_tensor(r5(t_by1), r5(t_by1), b5(t_gy1), ALU.min)
                nc.vector.tensor_sub(t_bx1[:], t_bx1[:], t_ix0[:])
                nc.vector.tensor_scalar_add(t_bx1[:], t_bx1[:], 1.0)
                nc.vector.tensor_sub(t_by1[:], t_by1[:], t_iy0[:])
                nc.vector.tensor_scalar_add(t_by1[:], t_by1[:], 1.0)
                nc.vector.tensor_mul(t_inter[:], t_bx1[:], t_by1[:])
                # a2 = (pw+1)*(ph+1); denom = a1 + a2 - inter
                nc.vector.tensor_scalar_add(t_pw[:], t_pw[:], 1.0)
                nc.vector.tensor_scalar_add(t_ph[:], t_ph[:], 1.0)
                nc.vector.tensor_mul(t_den[:], t_pw[:], t_ph[:])
                nc.vector.tensor_tensor(r5(t_den), r5(t_den), b5(t_a1), ALU.add)
                nc.vector.tensor_sub(t_den[:], t_den[:], t_inter[:])
                nc.vector.reciprocal(t_den[:], t_den[:])
                nc.vector.tensor_mul(t_iou[:], t_inter[:], t_den[:])

                # ---- argmax over anchors (first max wins) ----
                nc.vector.reduce_max(t_mm[:], r5(t_iou), axis=AX.X)
                nc.vector.tensor_tensor(
                    r5(t_scr), r5(t_iou), b5(t_mm), ALU.is_equal)
                nc.vector.tensor_tensor(
                    r5(t_scr2), r5(t_scr), c5(t_i5m), ALU.mult)
                nc.vector.tensor_reduce(
                    t_aidx[:], r5(t_scr2), axis=AX.X, op=ALU.min)
                nc.vector.tensor_scalar_add(t_aidx[:], t_aidx[:], 99.0)

                # ---- slot id s = 169*aidx + k (dedup deferred) ----
                nc.vector.scalar_tensor_tensor(
                    out=t_sid64[:, jsl], in0=t_aidx[:], scalar=float(CELLS),
                    in1=t_k[:, jsl], op0=ALU.mult, op1=ALU.add)

                # ---- anchor one-hot ----
                nc.vector.tensor_tensor(
                    r5(t_ohA), b5(t_aidx), c5(t_i5), ALU.is_equal)

                # ---- coord SSE, winner-selected ----
                dv = t_diff[:].rearrange("p (j a c) -> p j a c", a=NA, c=4)
                nc.vector.tensor_tensor(
                    dv[:, :, :, 0], q0, b5(t_tx[:, jsl]), ALU.subtract)
                nc.vector.tensor_tensor(
                    dv[:, :, :, 1], q1, b5(t_ty[:, jsl]), ALU.subtract)
                nc.vector.tensor_tensor(r5(t_scr), q2, c5(t_s2c), ALU.mult)
                nc.vector.tensor_tensor(
                    dv[:, :, :, 2], r5(t_scr), b5(w_ap[:, jsl]), ALU.subtract)
                nc.vector.tensor_tensor(r5(t_scr), q3, c5(t_s3c), ALU.mult)
                nc.vector.tensor_tensor(
                    dv[:, :, :, 3], r5(t_scr), b5(h_ap[:, jsl]), ALU.subtract)
                nc.vector.tensor_mul(t_diff[:], t_diff[:], t_diff[:])
                nc.vector.tensor_reduce(r5(t_csse), dv, axis=AX.X, op=ALU.add)
                nc.vector.tensor_mul(t_csse[:], t_csse[:], t_ohA[:])
                nc.vector.tensor_reduce(
                    tv[:, 0, jsl], r5(t_csse), axis=AX.X, op=ALU.add)

                # ---- conf terms at slots: (1-q4)^2 and q4^2, selected ----
                nc.vector.tensor_scalar(
                    r5(t_c1), q4, -1.0, 1.0, ALU.mult, ALU.add)
                nc.vector.tensor_mul(t_c1[:], t_c1[:], t_c1[:])
                nc.vector.tensor_mul(t_c1[:], t_c1[:], t_ohA[:])
                nc.vector.tensor_reduce(
                    tv[:, 1, jsl], r5(t_c1), axis=AX.X, op=ALU.add)
                nc.vector.tensor_tensor(r5(t_scr), q4, q4, ALU.mult)
                nc.vector.tensor_mul(t_scr[:], t_scr[:], t_ohA[:])
                nc.vector.tensor_reduce(
                    tv[:, 2, jsl], r5(t_scr), axis=AX.X, op=ALU.add)

                # ---- class term: sel_a sum_cls q*(q - 2*onehot20) ----
                ohv = t_oh[:].rearrange("p (j c) -> p j c", c=NCLS)
                nc.vector.tensor_tensor(
                    ohv,
                    t_cls[:, jsl].rearrange("p (j one) -> p j one", one=1)
                    .to_broadcast([128, JPP, NCLS]),
                    t_i20[:].rearrange("p (one c) -> p one c", one=1)
                    .to_broadcast([128, JPP, NCLS]),
                    ALU.is_equal)
                qcv = t_qcl[:].rearrange("p (j a c) -> p j a c", a=NA, c=NCLS)
                for a in range(NA):
                    nc.vector.scalar_tensor_tensor(
                        out=qcv[:, :, a, :], in0=ohv, scalar=-2.0,
                        in1=qclsv[:, :, a, :], op0=ALU.mult, op1=ALU.add)
                nc.vector.tensor_tensor(qcv, qcv, qclsv, ALU.mult)
                nc.vector.tensor_reduce(r5(t_scr), qcv, axis=AX.X, op=ALU.add)
                nc.vector.tensor_mul(t_scr[:], t_scr[:], t_ohA[:])
                nc.vector.tensor_reduce(
                    tv[:, 3, jsl], r5(t_scr), axis=AX.X, op=ALU.add)

            # ---- chunk pipeline (chunk 0 gathered in two halves) ----
            for c in range(NCHUNK):
                if c == 0:
                    nc.gpsimd.ap_gather(
                        out_ap=t_G0[:, 0:NIDX // 2],
                        in_ap=t_T0[:, 0:H * CELLS],
                        idxs_ap=t_idx16[:, 0:32],
                        channels=128, num_elems=H * CELLS, d=1,
                        num_idxs=NIDX // 2)
                    nc.gpsimd.ap_gather(
                        out_ap=t_G0[:, NIDX // 2:NIDX],
                        in_ap=t_T0[:],
                        idxs_ap=t_idx16[:, 32:64],
                        channels=128, num_elems=NE, d=1,
                        num_idxs=NIDX // 2)
                else:
                    gather_chunk(c)
                if c + 2 < NCHUNK:
                    load_chunk(c + 2)
                tr_copies(c)
                if c % 2 == 1:
                    pass_math(c // 2)

            # ---- dense conf channels: det[:, 25a+4, :] as [128, 2*169] ----
            for a in range(NA):
                src = _dram_ap(
                    det,
                    [[IMFLT, 128], [128 * IMFLT, 2], [1, CELLS]],
                    (a * CH + 4) * CELLS)
                nc.scalar.dma_start(
                    t_cf5[:, a * 2 * CELLS:(a + 1) * 2 * CELLS]
                    .rearrange("p (bh e) -> p bh e", e=CELLS), src)

            # ---- deferred last-writer-wins dedup over (aidx, cell) ----
            t_tps = psA.tile([J2, 128], f32, space="PSUM", tag="ded", bufs=1)
            nc.tensor.transpose(out=t_tps[:], in_=t_sid64[:], identity=t_id[:])
            nc.scalar.activation(t_sT[:], t_tps[:], ACT.Copy)
            for h in range(2):
                sl128 = slice(h * 64, (h + 1) * 64)
                sTa = t_sT[:, sl128].rearrange(
                    "p (bi o one) -> p bi o one", bi=2, one=1) \
                    .to_broadcast([J2, 2, O, O])
                sTb = t_sT[:, sl128].rearrange(
                    "p (bi one o2) -> p bi one o2", bi=2, one=1) \
                    .to_broadcast([J2, 2, O, O])
                eqv = t_eqp[:].rearrange("p (bi o o2) -> p bi o o2",
                                         bi=2, o2=O)
                nc.vector.tensor_tensor(eqv, sTa, sTb, ALU.is_equal)
                triv = t_tri[0:J2, :].rearrange(
                    "p (one o o2) -> p one o o2", one=1, o2=O) \
                    .to_broadcast([J2, 2, O, O])
                nc.vector.tensor_tensor(eqv, eqv, triv, ALU.mult)
                nc.vector.tensor_reduce(
                    t_dead[:, sl128].rearrange("p (bi o) -> p bi o", o=O),
                    eqv, axis=AX.X, op=ALU.max)
            t_tpw = psA.tile([128, J2], f32, space="PSUM", tag="ded2", bufs=1)
            nc.tensor.transpose(
                out=t_tpw[:], in_=t_dead[:], identity=t_id[0:J2, 0:J2])
            nc.scalar.activation(t_win[:], t_tpw[:], ACT.Copy)
            nc.vector.tensor_scalar(
                t_win[:], t_win[:], -1.0, 1.0, ALU.mult, ALU.add)

            # ---- masked accumulation into staging ----
            nc.vector.tensor_tensor(
                tv, tv,
                t_win[:].rearrange("p (one j) -> p one j", one=1)
                .to_broadcast([128, 4, J2]),
                ALU.mult)
            nc.vector.tensor_reduce(
                t_stage[:, 0:4].rearrange("p (t one) -> p t one", one=1),
                tv, axis=AX.X, op=ALU.add)
            nc.vector.reduce_sum(t_stage[:, 4:5], t_win[:], axis=AX.X)

            # ---- dense conf sum: square + reduce per anchor ----
            nc.vector.tensor_mul(t_cf5[:], t_cf5[:], t_cf5[:])
            nc.vector.tensor_reduce(
                t_stage[:, 5:10].rearrange("p (a one) -> p a one", one=1),
                t_cf5[:].rearrange("p (a e) -> p a e", a=NA),
                axis=AX.X, op=ALU.add)

            nc.sync.dma_start(out[:], t_stage[:])

    nc.compile()
    return nc


def _get_built():
    if "nc" not in _CACHE:
        _CACHE["nc"] = _build()
        _CACHE["consts"] = _make_consts()
    return _CACHE["nc"], _CACHE["consts"]


def _reduce_partials(P):
    """P: [ncores, 128, 16] fp32 partials -> the 4 scalar losses."""
    S = P.astype(np.float64).sum(axis=(0, 1))
    coord, confobj, confsub, clsq, wsum = S[0], S[1], S[2], S[3], S[4]
    dense = S[5:10].sum()
    obj_loss = 5.0 * coord + confobj
    no_obj_loss = 0.5 * (dense - confsub)
    conf_loss = clsq + wsum
    loss = obj_loss + no_obj_loss + conf_loss
    return (np.float32(loss), np.float32(obj_loss),
            np.float32(no_obj_loss), np.float32(conf_loss))


def _make_in_maps(detection_result, gt_boxes, gt_class, consts):
    det = np.ascontiguousarray(
        np.asarray(detection_result, dtype=np.float32)).reshape(B, -1)
    gtb = np.ascontiguousarray(np.asarray(gt_boxes, dtype=np.float32))
    clsf = np.asarray(gt_class).astype(np.float32)

    pad = np.zeros(DPAD, dtype=np.float32)
    in_maps = []
    for c in range(NCORES):
        sl = slice(c * BLOC, (c + 1) * BLOC)
        m = {"det": np.concatenate([det[sl].ravel(), pad]),
             "gtb": gtb[sl], "clsf": clsf[sl]}
        m.update(consts)
        in_maps.append(m)
    return in_maps


def kernel(detection_result, gt_boxes, gt_class):
    from concourse.bass_utils import run_bass_kernel_spmd

    nc, consts = _get_built()
    in_maps = _make_in_maps(detection_result, gt_boxes, gt_class, consts)
    res = run_bass_kernel_spmd(nc, in_maps, core_ids=list(range(NCORES)))
    P = np.stack([res.results[c]["out"] for c in range(NCORES)])
    return _reduce_partials(P)
